# revision 41
# baseline (speedup 1.0000x reference)
"""Mixtral-style GQA attention block on 8 Trainium2 NeuronCores.

Tensor-parallel over heads: core c owns q-heads [4c..4c+4) and kv-head c.
bf16 datapath throughout (2e-2 rel-err budget allows it): halves DMA
traffic, doubles DVE elementwise rate, enables FWL weight loads.

Per core:
  phase 1 (n-outer over 512-token chunks): qkv proj with full 32-step
    PSUM accumulation (no SBUF adds) -> RoPE (PE permutation rotate)
    -> v transpose.
  phase 2: causal attention per chunk (transposed-scores layout, shared
    stationaries), softmax-normalize, per-chunk bf16 AllGather of the
    normalized AV, and o_proj for this core's 512 output columns emitted
    per chunk so it overlaps later chunks' attention.
Host concatenates the per-core column slices.

Model dims (hardcoded): T=2048, HIDDEN=4096, H=32, KV=8, D=128.
"""

from contextlib import ExitStack

import numpy as np
import ml_dtypes

import concourse.bass_utils as _bu
import concourse.mybir as mybir
import concourse.tile as tile
from concourse import bacc
from concourse.bass_utils import run_bass_kernel_spmd

# ---- problem dims ----
T = 2048
HIDDEN = 4096
H = 32
KV = 8
D = 128
THETA = 10000.0
SCALE = D ** -0.5

CORES = 8
QH = H // CORES            # 4 q heads per core
SLOTS = QH + 2             # q0..q3, k, v head-major slots
LOCAL = QH * D             # 512: per-core attention output dims
P = 128
NCH = T // 512             # 4 token chunks of 512
KCH = HIDDEN // P          # 32 contraction chunks
TT = T // P                # 16 token tiles of 128

BF16 = mybir.dt.bfloat16
F32 = mybir.dt.float32
F32R = mybir.dt.float32r
EXP = mybir.ActivationFunctionType.Exp
BFNP = ml_dtypes.bfloat16


def build_nc():
    nc = bacc.Bacc(num_devices=CORES)

    # ---- per-core I/O (host pre-laid-out for contiguous DMA) ----
    hid_r = nc.declare_dram_parameter("hid_r", [NCH, P, KCH, 512], BF16,
                                      isOutput=False)
    w_qkv_r = nc.declare_dram_parameter("w_qkv_r", [P, KCH, SLOTS * P], BF16,
                                        isOutput=False)
    w_o_r = nc.declare_dram_parameter("w_o_r", [P, KCH, LOCAL], BF16,
                                      isOutput=False)
    cosT = nc.declare_dram_parameter("cosT", [P, T], BF16, isOutput=False)
    sinT = nc.declare_dram_parameter("sinT", [P, T], BF16, isOutput=False)
    outT = nc.declare_dram_parameter("outT", [LOCAL, T], F32, isOutput=True)

    # ---- consts ----
    rotm = np.zeros((P, P), dtype=np.float32)
    rotm[np.arange(P), (np.arange(P) + 64) % P] = 1.0   # rot(x)[i] = x[(i+64)%128]
    rot_c = nc.inline_tensor(
        np.ascontiguousarray(rotm.T).astype(BFNP), name="rot_c")
    eye_c = nc.inline_tensor(np.eye(P, dtype=np.float32).astype(BFNP),
                             name="eye_c")
    tri_c = nc.inline_tensor(
        np.triu(np.ones((P, P), dtype=np.float32)).astype(BFNP), name="tri_c")
    # dn lhsT for head h: [128, 4] with column h all-ones
    onc4 = np.zeros((P, QH, QH), dtype=np.float32)
    for h in range(QH):
        onc4[:, h, h] = 1.0
    onc4_c = nc.inline_tensor(
        np.ascontiguousarray(onc4.transpose(1, 0, 2)).astype(BFNP),
        name="onc4_c")   # [QH, 128, 4]
    ones_row_c = nc.inline_tensor(np.ones((1, P), dtype=np.float32).astype(BFNP),
                                  name="ones_row_c")

    # ---- collective bounce buffers (bf16) ----
    # chunks are AllGathered in PAIRS (0+1, 2+3): each collective costs
    # ~31us of fixed trigger+handshake overhead on the serialized CC
    # stream, so two big AGs beat four small ones. Gather output is
    # core-major: rows [g*1024 + c_in_pair*512 + d] for core g.
    ag_in = nc.dram_tensor("ag_in", [NCH, LOCAL, 512], BF16)
    ag_out = nc.dram_tensor("ag_out", [2, 2 * H * D, 512], BF16,
                            addr_space="Shared")


    with tile.TileContext(nc) as tc:
        with tc.tile_pool(name="const", bufs=1) as cpool:
            qstack = ExitStack()
            qpool = qstack.enter_context(tc.tile_pool(name="qkv_out", bufs=1))
            rot_sb = cpool.tile([P, P], BF16, tag="rot")
            eye_sb = cpool.tile([P, P], BF16, tag="eye")
            tri_sb = cpool.tile([P, P], BF16, tag="tri")
            onc4_sb = [cpool.tile([P, QH], BF16, tag=f"onc4_{h}",
                                  name=f"onc4_{h}")
                       for h in range(QH)]
            onrb_sb = cpool.tile([1, P], BF16, tag="onrb")
            nc.sync.dma_start(rot_sb[:], rot_c[:, :])
            nc.sync.dma_start(eye_sb[:], eye_c[:, :])
            nc.sync.dma_start(tri_sb[:], tri_c[:, :])
            for h in range(QH):
                nc.sync.dma_start(onc4_sb[h][:], onc4_c[h])
            nc.sync.dma_start(onrb_sb[:], ones_row_c[:, :])


            # persistent qkv outputs (head-major); rope applied in place
            qkv_sb = [qpool.tile([P, T], BF16, tag=f"qkv{m}", name=f"qkv{m}")
                      for m in range(SLOTS)]
            q_rope = qkv_sb[:QH]
            k_rope = qkv_sb[QH]
            vtok = [qpool.tile([P, P], BF16, tag=f"vt{j}", name=f"vt{j}")
                    for j in range(TT)]

            # ============ phase 1: qkv proj (full-psum accum) + rope ====
            # kc-outer / m-inner over 6 concurrent PSUM banks: PE starts as
            # soon as the first contraction chunk lands, instead of waiting
            # for the full weight+hid preload.
            with tc.tile_pool(name="wq", bufs=1) as wq_pool, \
                 tc.tile_pool(name="hid", bufs=2) as hid_pool, \
                 tc.tile_pool(name="cs", bufs=1) as cs_pool, \
                 tc.tile_pool(name="ev", bufs=4) as ev_pool, \
                 tc.tile_pool(name="qkv_ps", bufs=1, space="PSUM") as qkv_ps, \
                 tc.tile_pool(name="rot_ps", bufs=2, space="PSUM") as rot_ps:
                wq = wq_pool.tile([P, KCH, SLOTS * P], BF16, tag="wq")
                for kc in range(KCH):
                    nc.sync.dma_start(wq[:, kc, :], w_qkv_r[:, kc, :])
                cos_sb = cs_pool.tile([P, T], BF16, tag="cos")
                sin_sb = cs_pool.tile([P, T], BF16, tag="sin")
                for n in range(NCH):
                    nc.sync.dma_start(cos_sb[:, n * 512:(n + 1) * 512],
                                      cosT[:, n * 512:(n + 1) * 512])
                    nc.sync.dma_start(sin_sb[:, n * 512:(n + 1) * 512],
                                      sinT[:, n * 512:(n + 1) * 512])

                for n in range(NCH):
                    t0 = n * 512
                    ht = hid_pool.tile([P, KCH, 512], BF16, tag="ht",
                                       name="ht")
                    for kc in range(KCH):
                        nc.scalar.dma_start(ht[:, kc, :],
                                            hid_r[n, :, kc, :])
                    pss = [qkv_ps.tile([P, 512], F32, tag=f"qp{m}",
                                       name=f"qp{m}")
                           for m in range(SLOTS)]
                    for kc in range(KCH):
                        for m in range(SLOTS):
                            nc.tensor.matmul(
                                pss[m][:], wq[:, kc, m * P:(m + 1) * P],
                                ht[:, kc, :],
                                start=(kc == 0), stop=(kc == KCH - 1),
                                skip_group_check=True)
                    for m in range(SLOTS):
                        dslc = qkv_sb[m][:, t0:t0 + 512]
                        nc.vector.tensor_copy(dslc, pss[m][:])
                        if m < SLOTS - 1:
                            # neox rope via PE permutation rotate
                            rps = rot_ps.tile([P, 512], F32, tag="rot",
                                              name="rps")
                            nc.tensor.matmul(rps[:], rot_sb[:], dslc,
                                             start=True, stop=True)
                            tmp = ev_pool.tile([P, 512], BF16, tag="tmp")
                            nc.vector.tensor_mul(tmp[:], rps[:],
                                                 sin_sb[:, t0:t0 + 512])
                            nc.vector.tensor_mul(dslc, dslc,
                                                 cos_sb[:, t0:t0 + 512])
                            nc.vector.tensor_add(dslc, dslc, tmp[:])
                        else:
                            # v: transpose to [keys, D] token tiles
                            for jj in range(4):
                                j = n * 4 + jj
                                # bf16 [P,1024] = same 2KB bank footprint as
                                # f32 [P,512]; transpose out dtype must match
                                # its bf16 input
                                tps = rot_ps.tile([P, 1024], BF16, tag="rot",
                                                  name="tps")
                                nc.tensor.transpose(
                                    tps[:, :P],
                                    qkv_sb[SLOTS - 1][:, j * P:(j + 1) * P],
                                    eye_sb[:])
                                nc.scalar.copy(vtok[j][:], tps[:, :P])

            # ============ phase 2: attention + chunked AllGather + o_proj ====
            with tc.tile_pool(name="wo", bufs=1) as wo_pool, \
                 tc.tile_pool(name="att", bufs=1) as att_pool, \
                 tc.tile_pool(name="sm", bufs=2) as sm_pool, \
                 tc.tile_pool(name="agr", bufs=3) as ag_pool, \
                 tc.tile_pool(name="ob", bufs=2) as out_pool, \
                 tc.tile_pool(name="sc_ps", bufs=2, space="PSUM") as sc_ps, \
                 tc.tile_pool(name="av_ps", bufs=1, space="PSUM") as av_ps, \
                 tc.tile_pool(name="dn_ps", bufs=1, space="PSUM") as dn_ps, \
                 tc.tile_pool(name="op_ps", bufs=1, space="PSUM") as op_ps:
                # wo + ag reads ride the GpSimd queue: an ag read waits on
                # its AllGather's completion, and a waiting DMA blocks the
                # whole issuing engine's FIFO — on Sync that wedged the
                # norm/dnr DMAs of later chunks behind it.
                wo_sb = wo_pool.tile([P, KCH, LOCAL], BF16, tag="wo")
                for kc in range(KCH):
                    nc.gpsimd.dma_start(wo_sb[:, kc, :], w_o_r[:, kc, :])

                def make_oproj(c):
                    # split into pieces so the PE-queue (strict MATMUL FIFO)
                    # never holds an oproj MM that still waits on its
                    # AllGather: pieces are emitted two chunks later, when
                    # AG(c) has long completed.
                    state = {}

                    def load_ag():
                        ag = ag_pool.tile([P, KCH, 512], BF16, tag="ag",
                                          name="ag")
                        for kb in range(KCH // 4):
                            r0 = kb * 1024 + (c % 2) * 512
                            nc.gpsimd.dma_start(
                                ag[:, kb * 4:(kb + 1) * 4, :],
                                ag_out[c // 2, r0:r0 + 512, :]
                                .rearrange("(kc p) t -> p kc t", p=P))
                        state["ag"] = ag

                    def make_m(m):
                        def mpiece():
                            ag = state["ag"]
                            ps = op_ps.tile([P, 512], F32, tag="op",
                                            name="op")
                            for kc in range(KCH):
                                nc.tensor.matmul(
                                    ps[:], wo_sb[:, kc, m * P:(m + 1) * P],
                                    ag[:, kc, :],
                                    start=(kc == 0), stop=(kc == KCH - 1),
                                    skip_group_check=True)
                            ob = out_pool.tile([P, 512], F32, tag="ob")
                            nc.vector.tensor_copy(ob[:], ps[:])
                            nc.sync.dma_start(
                                outT[m * P:(m + 1) * P,
                                     c * 512:(c + 1) * 512], ob[:])
                        return mpiece

                    return load_ag, [make_m(m) for m in range(LOCAL // P)]

                oproj = []

                norm_pending = None
                for c in range(NCH):
                    t0 = c * 512
                    jmax = 4 * c + 3
                    avp = [av_ps.tile([P, 512], F32, tag=f"av{h}",
                                      name=f"av{h}")
                           for h in range(QH)]
                    dnp = dn_ps.tile([QH, 512], F32, tag="dn")
                    atts = {}

                    def scores(j, c=c, t0=t0, atts=atts):
                        toff = max(t0, j * P)
                        w = t0 + 512 - toff
                        for h in range(QH):
                            scp = sc_ps.tile([P, 512], F32, tag="sc",
                                             name="scp")
                            nc.tensor.matmul(
                                scp[:, :w], k_rope[:, j * P:(j + 1) * P],
                                q_rope[h][:, toff:toff + w],
                                start=True, stop=True)
                            att = att_pool.tile([P, 512], BF16, tag="att",
                                                name="att", bufs=10)
                            nc.scalar.activation(att[:, :w], scp[:, :w], EXP,
                                                 scale=SCALE)
                            if j >= 4 * c:  # diagonal block: causal mask
                                nc.vector.tensor_mul(att[:, :P], att[:, :P],
                                                     tri_sb[:])
                            atts[(j, h)] = (att, toff, w)

                    def avdn(j, c=c, t0=t0, jmax=jmax, atts=atts, avp=avp,
                             dnp=dnp):
                        for h in range(QH):
                            att, toff, w = atts[(j, h)]
                            o = toff - t0
                            nc.tensor.matmul(
                                avp[h][:, o:o + w], vtok[j][:], att[:, :w],
                                start=(j == 0), stop=(j == jmax),
                                skip_group_check=True)
                        for h in range(QH):
                            att, toff, w = atts[(j, h)]
                            o = toff - t0
                            nc.tensor.matmul(
                                dnp[:, o:o + w], onc4_sb[h][:], att[:, :w],
                                start=(j == 0 and h == 0),
                                stop=(j == jmax and h == QH - 1),
                                skip_group_check=True)

                    def make_norm(c=c, avp=avp, dnp=dnp):
                        def norm():
                            dn_sb = sm_pool.tile([QH, 512], F32, tag="dn_sb")
                            nc.scalar.copy(dn_sb[:], dnp[:])
                            rc4 = sm_pool.tile([QH, 512], F32, tag="rc4")
                            scr = sm_pool.tile([QH, 512], F32, tag="scr")
                            nc.vector.reciprocal_approx_accurate(
                                rc4[:], dn_sb[:], scr[:])
                            # bf16 path keeps the broadcast MM single-pass
                            # (f32 runs the PE in two-pass LOW/HIGH mode)
                            rc4b = sm_pool.tile([QH, 512], BF16, tag="rc4b")
                            nc.vector.tensor_copy(rc4b[:], rc4[:])
                            # one partition-collapsing DMA for all 4 heads
                            dnr4 = sm_pool.tile([1, QH, 512], BF16,
                                                tag="dnr4", bufs=2)
                            nc.sync.dma_start(dnr4[:], rc4b[:])
                            for h in range(QH):
                                bcp = sc_ps.tile([P, 512], F32, tag="sc",
                                                 name="bcp")
                                nc.tensor.matmul(bcp[:], onrb_sb[:],
                                                 dnr4[0:1, h, :],
                                                 start=True, stop=True)
                                bc_sb = sm_pool.tile([P, 512], F32,
                                                     tag="bc_sb", bufs=4)
                                nc.scalar.copy(bc_sb[:], bcp[:])
                                avn = sm_pool.tile([P, 512], BF16, tag="avn",
                                                   bufs=4)
                                nc.vector.tensor_mul(avn[:], avp[h][:],
                                                     bc_sb[:])
                                nc.sync.dma_start(
                                    ag_in[c, h * P:(h + 1) * P, :], avn[:])
                            if c % 2 == 1:
                                nc.gpsimd.collective_compute(
                                    "AllGather",
                                    mybir.AluOpType.bypass,
                                    replica_groups=[list(range(CORES))],
                                    ins=[ag_in[c - 1:c + 1]],
                                    outs=[ag_out[c // 2]],
                                )
                        return norm

                    # software-pipeline: scores one j ahead; chunk c-1's
                    # normalization fires at j==0. All o_proj work is packed
                    # at the very end of the attention stream: the AGs
                    # complete underneath it, and no PE matmul ever waits in
                    # the strict PE FIFO on an unfinished collective. ag
                    # loads are placed so they never sit in the gpsimd FIFO
                    # ahead of a later AG trigger (they would delay it).
                    scores(0)
                    for j in range(jmax + 1):
                        if j < jmax:
                            scores(j + 1)
                        if j == 0 and norm_pending is not None:
                            norm_pending()
                        if c == NCH - 1:
                            if j == 1:
                                oproj[0][0]()        # load ag(0)
                            elif j == 8:
                                oproj[1][0]()        # load ag(1)
                            elif j >= 12:
                                oproj[0][1][j - 12]()   # mp(0) pieces
                        avdn(j)
                    norm_pending = make_norm()
                    oproj.append(make_oproj(c))
                norm_pending()                        # issues AG(3)
                oproj[2][0]()                         # load ag(2)
                oproj[3][0]()                         # load ag(3): parks on
                                                      # AG(3), queue empty
                for c2 in (1, 2, 3):
                    for piece in oproj[c2][1]:
                        piece()

            qstack.close()

    nc.finalize()
    return nc


_NC_CACHE = None


def _host_prep(positions, hidden_states, w_qkv, w_o):
    pos = np.asarray(positions).astype(np.float64)
    half = D // 2
    inv_freq = 1.0 / (THETA ** (np.arange(half, dtype=np.float64) * 2.0 / D))
    freqs = pos[:, None] * inv_freq[None, :]          # [T, 64]
    cos = np.cos(freqs).astype(np.float32).T          # [64, T]
    sin = np.sin(freqs).astype(np.float32).T
    cosT = np.ascontiguousarray(
        np.concatenate([cos, cos], axis=0)).astype(BFNP)    # [128, T]
    sinT = np.ascontiguousarray(
        np.concatenate([-sin, sin], axis=0)).astype(BFNP)   # sign fold
    hs = np.asarray(hidden_states, dtype=np.float32)
    # hid_r[n, p, kc, t] = hs[n*512+t, kc*128+p]
    hid_r = np.ascontiguousarray(
        hs.reshape(NCH, 512, KCH, P).transpose(0, 3, 2, 1)).astype(BFNP)
    w_qkv = np.asarray(w_qkv, dtype=np.float32)
    w_o = np.asarray(w_o, dtype=np.float32)

    in_maps = []
    for c in range(CORES):
        rows = [
            w_qkv[c * QH * D:(c + 1) * QH * D],                         # 4 q heads
            w_qkv[H * D + c * D: H * D + (c + 1) * D],                  # k head
            w_qkv[(H + KV) * D + c * D: (H + KV) * D + (c + 1) * D],    # v head
        ]
        wq_c = np.concatenate(rows, axis=0)                  # [768, 4096]
        # w_qkv_r[p, kc, m] = wq_c[m, kc*128+p]
        w_qkv_r = np.ascontiguousarray(
            wq_c.reshape(SLOTS * P, KCH, P).transpose(2, 1, 0)).astype(BFNP)
        wo_cT = w_o[c * LOCAL:(c + 1) * LOCAL, :].T          # [4096, 512]
        # w_o_r[p, kc, m] = wo_cT[kc*128+p, m]
        w_o_r = np.ascontiguousarray(
            wo_cT.reshape(KCH, P, LOCAL).transpose(1, 0, 2)).astype(BFNP)
        in_maps.append({
            "hid_r": hid_r,
            "w_qkv_r": w_qkv_r,
            "w_o_r": w_o_r,
            "cosT": cosT,
            "sinT": sinT,
        })
    return in_maps


def kernel(positions, hidden_states, w_qkv, w_o):
    global _NC_CACHE
    in_maps = _host_prep(positions, hidden_states, w_qkv, w_o)
    if _NC_CACHE is None:
        _NC_CACHE = build_nc()
    res = None
    for attempt in range(3):
        try:
            res = run_bass_kernel_spmd(_NC_CACHE, in_maps,
                                       core_ids=list(range(CORES)))
            break
        except Exception:
            if attempt == 2:
                raise
    outs = [res.results[c]["outT"].T for c in range(CORES)]   # [2048, 512] each
    return np.ascontiguousarray(np.concatenate(outs, axis=1))


# revision 43
# speedup vs baseline: 1.0242x; 1.0242x over previous
"""Mixtral-style GQA attention block on 8 Trainium2 NeuronCores.

Tensor-parallel over heads: core c owns q-heads [4c..4c+4) and kv-head c.
bf16 datapath throughout (2e-2 rel-err budget allows it): halves DMA
traffic, doubles DVE elementwise rate, enables FWL weight loads.

Per core:
  phase 1 (n-outer over 512-token chunks): qkv proj with full 32-step
    PSUM accumulation (no SBUF adds) -> RoPE (PE permutation rotate)
    -> v transpose.
  phase 2: causal attention per chunk (transposed-scores layout, shared
    stationaries), softmax-normalize, per-chunk bf16 AllGather of the
    normalized AV, and o_proj for this core's 512 output columns emitted
    per chunk so it overlaps later chunks' attention.
Host concatenates the per-core column slices.

Model dims (hardcoded): T=2048, HIDDEN=4096, H=32, KV=8, D=128.
"""

from contextlib import ExitStack

import numpy as np
import ml_dtypes

import concourse.bass_utils as _bu
import concourse.mybir as mybir
import concourse.tile as tile
from concourse import bacc
from concourse.bass_utils import run_bass_kernel_spmd

# ---- problem dims ----
T = 2048
HIDDEN = 4096
H = 32
KV = 8
D = 128
THETA = 10000.0
SCALE = D ** -0.5

CORES = 8
QH = H // CORES            # 4 q heads per core
SLOTS = QH + 2             # q0..q3, k, v head-major slots
LOCAL = QH * D             # 512: per-core attention output dims
P = 128
NCH = T // 512             # 4 token chunks of 512
KCH = HIDDEN // P          # 32 contraction chunks
TT = T // P                # 16 token tiles of 128

BF16 = mybir.dt.bfloat16
F32 = mybir.dt.float32
F32R = mybir.dt.float32r
EXP = mybir.ActivationFunctionType.Exp
BFNP = ml_dtypes.bfloat16


def build_nc():
    nc = bacc.Bacc(num_devices=CORES)

    # ---- per-core I/O (host pre-laid-out for contiguous DMA) ----
    hid_r = nc.declare_dram_parameter("hid_r", [NCH, P, KCH, 512], BF16,
                                      isOutput=False)
    w_qkv_r = nc.declare_dram_parameter("w_qkv_r", [P, KCH, SLOTS * P], BF16,
                                        isOutput=False)
    w_o_r = nc.declare_dram_parameter("w_o_r", [P, KCH, LOCAL], BF16,
                                      isOutput=False)
    cosT = nc.declare_dram_parameter("cosT", [P, T], BF16, isOutput=False)
    sinT = nc.declare_dram_parameter("sinT", [P, T], BF16, isOutput=False)
    outT = nc.declare_dram_parameter("outT", [LOCAL, T], F32, isOutput=True)

    # ---- consts ----
    rotm = np.zeros((P, P), dtype=np.float32)
    rotm[np.arange(P), (np.arange(P) + 64) % P] = 1.0   # rot(x)[i] = x[(i+64)%128]
    rot_c = nc.inline_tensor(
        np.ascontiguousarray(rotm.T).astype(BFNP), name="rot_c")
    eye_c = nc.inline_tensor(np.eye(P, dtype=np.float32).astype(BFNP),
                             name="eye_c")
    tri_c = nc.inline_tensor(
        np.triu(np.ones((P, P), dtype=np.float32)).astype(BFNP), name="tri_c")
    # dn lhsT for head h: [128, 4] with column h all-ones
    onc4 = np.zeros((P, QH, QH), dtype=np.float32)
    for h in range(QH):
        onc4[:, h, h] = 1.0
    onc4_c = nc.inline_tensor(
        np.ascontiguousarray(onc4.transpose(1, 0, 2)).astype(BFNP),
        name="onc4_c")   # [QH, 128, 4]
    ones_row_c = nc.inline_tensor(np.ones((1, P), dtype=np.float32).astype(BFNP),
                                  name="ones_row_c")

    # ---- collective bounce buffers (bf16) ----
    # chunks are AllGathered in PAIRS (0+1, 2+3): each collective costs
    # ~31us of fixed trigger+handshake overhead on the serialized CC
    # stream, so two big AGs beat four small ones. Gather output is
    # core-major: rows [g*1024 + c_in_pair*512 + d] for core g.
    ag_in = nc.dram_tensor("ag_in", [NCH, LOCAL, 512], BF16)
    ag_out = nc.dram_tensor("ag_out", [2, 2 * H * D, 512], BF16,
                            addr_space="Shared")


    with tile.TileContext(nc) as tc:
        with tc.tile_pool(name="const", bufs=1) as cpool:
            qstack = ExitStack()
            qpool = qstack.enter_context(tc.tile_pool(name="qkv_out", bufs=1))
            rot_sb = cpool.tile([P, P], BF16, tag="rot")
            eye_sb = cpool.tile([P, P], BF16, tag="eye")
            tri_sb = cpool.tile([P, P], BF16, tag="tri")
            onc4_sb = [cpool.tile([P, QH], BF16, tag=f"onc4_{h}",
                                  name=f"onc4_{h}")
                       for h in range(QH)]
            onrb_sb = cpool.tile([1, P], BF16, tag="onrb")
            nc.sync.dma_start(rot_sb[:], rot_c[:, :])
            nc.sync.dma_start(eye_sb[:], eye_c[:, :])
            nc.sync.dma_start(tri_sb[:], tri_c[:, :])
            for h in range(QH):
                nc.sync.dma_start(onc4_sb[h][:], onc4_c[h])
            nc.sync.dma_start(onrb_sb[:], ones_row_c[:, :])


            # persistent qkv outputs (head-major); rope applied in place
            qkv_sb = [qpool.tile([P, T], BF16, tag=f"qkv{m}", name=f"qkv{m}")
                      for m in range(SLOTS)]
            q_rope = qkv_sb[:QH]
            k_rope = qkv_sb[QH]
            vtok = [qpool.tile([P, P], BF16, tag=f"vt{j}", name=f"vt{j}")
                    for j in range(TT)]

            # ============ phase 1: qkv proj (full-psum accum) + rope ====
            # kc-outer / m-inner over 6 concurrent PSUM banks: PE starts as
            # soon as the first contraction chunk lands, instead of waiting
            # for the full weight+hid preload.
            with tc.tile_pool(name="wq", bufs=1) as wq_pool, \
                 tc.tile_pool(name="hid", bufs=2) as hid_pool, \
                 tc.tile_pool(name="cs", bufs=1) as cs_pool, \
                 tc.tile_pool(name="ev", bufs=4) as ev_pool, \
                 tc.tile_pool(name="qkv_ps", bufs=1, space="PSUM") as qkv_ps, \
                 tc.tile_pool(name="rot_ps", bufs=2, space="PSUM") as rot_ps:
                wq = wq_pool.tile([P, KCH, SLOTS * P], BF16, tag="wq")
                for kc in range(KCH):
                    nc.sync.dma_start(wq[:, kc, :], w_qkv_r[:, kc, :])
                cos_sb = cs_pool.tile([P, T], BF16, tag="cos")
                sin_sb = cs_pool.tile([P, T], BF16, tag="sin")
                for n in range(NCH):
                    nc.sync.dma_start(cos_sb[:, n * 512:(n + 1) * 512],
                                      cosT[:, n * 512:(n + 1) * 512])
                    nc.sync.dma_start(sin_sb[:, n * 512:(n + 1) * 512],
                                      sinT[:, n * 512:(n + 1) * 512])

                for n in range(NCH):
                    t0 = n * 512
                    ht = hid_pool.tile([P, KCH, 512], BF16, tag="ht",
                                       name="ht")
                    for kc in range(KCH):
                        nc.scalar.dma_start(ht[:, kc, :],
                                            hid_r[n, :, kc, :])
                    pss = [qkv_ps.tile([P, 512], F32, tag=f"qp{m}",
                                       name=f"qp{m}")
                           for m in range(SLOTS)]
                    for kc in range(KCH):
                        for m in range(SLOTS):
                            nc.tensor.matmul(
                                pss[m][:], wq[:, kc, m * P:(m + 1) * P],
                                ht[:, kc, :],
                                start=(kc == 0), stop=(kc == KCH - 1),
                                skip_group_check=True)
                    for m in range(SLOTS):
                        dslc = qkv_sb[m][:, t0:t0 + 512]
                        nc.vector.tensor_copy(dslc, pss[m][:])
                        if m < SLOTS - 1:
                            # neox rope via PE permutation rotate
                            rps = rot_ps.tile([P, 512], F32, tag="rot",
                                              name="rps")
                            nc.tensor.matmul(rps[:], rot_sb[:], dslc,
                                             start=True, stop=True)
                            tmp = ev_pool.tile([P, 512], BF16, tag="tmp")
                            nc.vector.tensor_mul(tmp[:], rps[:],
                                                 sin_sb[:, t0:t0 + 512])
                            nc.vector.tensor_mul(dslc, dslc,
                                                 cos_sb[:, t0:t0 + 512])
                            nc.vector.tensor_add(dslc, dslc, tmp[:])
                        else:
                            # v: transpose to [keys, D] token tiles
                            for jj in range(4):
                                j = n * 4 + jj
                                # bf16 [P,1024] = same 2KB bank footprint as
                                # f32 [P,512]; transpose out dtype must match
                                # its bf16 input
                                tps = rot_ps.tile([P, 1024], BF16, tag="rot",
                                                  name="tps")
                                nc.tensor.transpose(
                                    tps[:, :P],
                                    qkv_sb[SLOTS - 1][:, j * P:(j + 1) * P],
                                    eye_sb[:])
                                nc.scalar.copy(vtok[j][:], tps[:, :P])

            # ============ phase 2: attention + chunked AllGather + o_proj ====
            with tc.tile_pool(name="wo", bufs=1) as wo_pool, \
                 tc.tile_pool(name="att", bufs=1) as att_pool, \
                 tc.tile_pool(name="sm", bufs=2) as sm_pool, \
                 tc.tile_pool(name="agr", bufs=3) as ag_pool, \
                 tc.tile_pool(name="ob", bufs=2) as out_pool, \
                 tc.tile_pool(name="sc_ps", bufs=2, space="PSUM") as sc_ps, \
                 tc.tile_pool(name="av_ps", bufs=1, space="PSUM") as av_ps, \
                 tc.tile_pool(name="dn_ps", bufs=1, space="PSUM") as dn_ps, \
                 tc.tile_pool(name="op_ps", bufs=1, space="PSUM") as op_ps:
                # wo + ag reads ride the GpSimd queue: an ag read waits on
                # its AllGather's completion, and a waiting DMA blocks the
                # whole issuing engine's FIFO — on Sync that wedged the
                # norm/dnr DMAs of later chunks behind it.
                wo_sb = wo_pool.tile([P, KCH, LOCAL], BF16, tag="wo")
                for kc in range(KCH):
                    nc.gpsimd.dma_start(wo_sb[:, kc, :], w_o_r[:, kc, :])

                def make_oproj(c):
                    # split into pieces so the PE-queue (strict MATMUL FIFO)
                    # never holds an oproj MM that still waits on its
                    # AllGather: pieces are emitted two chunks later, when
                    # AG(c) has long completed.
                    state = {}

                    def load_ag():
                        ag = ag_pool.tile([P, KCH, 512], BF16, tag="ag",
                                          name="ag")
                        for kb in range(KCH // 4):
                            r0 = kb * 1024 + (c % 2) * 512
                            nc.gpsimd.dma_start(
                                ag[:, kb * 4:(kb + 1) * 4, :],
                                ag_out[c // 2, r0:r0 + 512, :]
                                .rearrange("(kc p) t -> p kc t", p=P))
                        state["ag"] = ag

                    def make_m(m):
                        def mpiece():
                            ag = state["ag"]
                            ps = op_ps.tile([P, 512], F32, tag="op",
                                            name="op")
                            for kc in range(KCH):
                                nc.tensor.matmul(
                                    ps[:], wo_sb[:, kc, m * P:(m + 1) * P],
                                    ag[:, kc, :],
                                    start=(kc == 0), stop=(kc == KCH - 1),
                                    skip_group_check=True)
                            ob = out_pool.tile([P, 512], F32, tag="ob")
                            nc.vector.tensor_copy(ob[:], ps[:])
                            nc.sync.dma_start(
                                outT[m * P:(m + 1) * P,
                                     c * 512:(c + 1) * 512], ob[:])
                        return mpiece

                    return load_ag, [make_m(m) for m in range(LOCAL // P)]

                oproj = []

                norm_pending = None
                for c in range(NCH):
                    t0 = c * 512
                    jmax = 4 * c + 3
                    avp = [av_ps.tile([P, 512], F32, tag=f"av{h}",
                                      name=f"av{h}")
                           for h in range(QH)]
                    dnp = dn_ps.tile([QH, 512], F32, tag="dn")
                    atts = {}

                    def scores(j, c=c, t0=t0, atts=atts):
                        toff = max(t0, j * P)
                        w = t0 + 512 - toff
                        for h in range(QH):
                            scp = sc_ps.tile([P, 512], F32, tag="sc",
                                             name="scp")
                            nc.tensor.matmul(
                                scp[:, :w], k_rope[:, j * P:(j + 1) * P],
                                q_rope[h][:, toff:toff + w],
                                start=True, stop=True)
                            att = att_pool.tile([P, 512], BF16, tag="att",
                                                name="att", bufs=10)
                            nc.scalar.activation(att[:, :w], scp[:, :w], EXP,
                                                 scale=SCALE)
                            if j >= 4 * c:  # diagonal block: causal mask
                                nc.vector.tensor_mul(att[:, :P], att[:, :P],
                                                     tri_sb[:])
                            atts[(j, h)] = (att, toff, w)

                    def avdn(j, c=c, t0=t0, jmax=jmax, atts=atts, avp=avp,
                             dnp=dnp):
                        for h in range(QH):
                            att, toff, w = atts[(j, h)]
                            o = toff - t0
                            nc.tensor.matmul(
                                avp[h][:, o:o + w], vtok[j][:], att[:, :w],
                                start=(j == 0), stop=(j == jmax),
                                skip_group_check=True)
                        for h in range(QH):
                            att, toff, w = atts[(j, h)]
                            o = toff - t0
                            nc.tensor.matmul(
                                dnp[:, o:o + w], onc4_sb[h][:], att[:, :w],
                                start=(j == 0 and h == 0),
                                stop=(j == jmax and h == QH - 1),
                                skip_group_check=True)

                    def make_norm(c=c, avp=avp, dnp=dnp):
                        def norm():
                            dn_sb = sm_pool.tile([QH, 512], F32, tag="dn_sb")
                            nc.scalar.copy(dn_sb[:], dnp[:])
                            rc4 = sm_pool.tile([QH, 512], F32, tag="rc4")
                            scr = sm_pool.tile([QH, 512], F32, tag="scr")
                            nc.vector.reciprocal_approx_accurate(
                                rc4[:], dn_sb[:], scr[:])
                            # bf16 path keeps the broadcast MM single-pass
                            # (f32 runs the PE in two-pass LOW/HIGH mode)
                            rc4b = sm_pool.tile([QH, 512], BF16, tag="rc4b")
                            nc.vector.tensor_copy(rc4b[:], rc4[:])
                            # one partition-collapsing DMA for all 4 heads
                            dnr4 = sm_pool.tile([1, QH, 512], BF16,
                                                tag="dnr4", bufs=2)
                            nc.sync.dma_start(dnr4[:], rc4b[:])
                            for h in range(QH):
                                bcp = sc_ps.tile([P, 512], F32, tag="sc",
                                                 name="bcp")
                                nc.tensor.matmul(bcp[:], onrb_sb[:],
                                                 dnr4[0:1, h, :],
                                                 start=True, stop=True)
                                bc_sb = sm_pool.tile([P, 512], F32,
                                                     tag="bc_sb", bufs=4)
                                nc.scalar.copy(bc_sb[:], bcp[:])
                                avn = sm_pool.tile([P, 512], BF16, tag="avn",
                                                   bufs=4)
                                nc.vector.tensor_mul(avn[:], avp[h][:],
                                                     bc_sb[:])
                                nc.sync.dma_start(
                                    ag_in[c, h * P:(h + 1) * P, :], avn[:])
                            if c % 2 == 1:
                                nc.gpsimd.collective_compute(
                                    "AllGather",
                                    mybir.AluOpType.bypass,
                                    replica_groups=[list(range(CORES))],
                                    ins=[ag_in[c - 1:c + 1]],
                                    outs=[ag_out[c // 2]],
                                )
                        return norm

                    # software-pipeline: scores one j ahead; chunk c-1's
                    # normalization fires at j==0. All o_proj work is packed
                    # at the very end of the attention stream: the AGs
                    # complete underneath it, and no PE matmul ever waits in
                    # the strict PE FIFO on an unfinished collective. ag
                    # loads are placed so they never sit in the gpsimd FIFO
                    # ahead of a later AG trigger (they would delay it).
                    scores(0)
                    for j in range(jmax + 1):
                        if j < jmax:
                            scores(j + 1)
                        if j == 0 and norm_pending is not None:
                            norm_pending()
                        if c == NCH - 1:
                            if j == 1:
                                oproj[0][0]()        # load ag(0)
                            elif j == 8:
                                oproj[1][0]()        # load ag(1)
                        avdn(j)
                    norm_pending = make_norm()
                    oproj.append(make_oproj(c))
                norm_pending()                        # issues AG(23)
                oproj[2][0]()                         # load ag(2)
                oproj[3][0]()                         # load ag(3)
                for c2 in (0, 1, 2, 3):
                    for piece in oproj[c2][1]:
                        piece()

            qstack.close()

    nc.finalize()
    return nc


_NC_CACHE = None


def _host_prep(positions, hidden_states, w_qkv, w_o):
    pos = np.asarray(positions).astype(np.float64)
    half = D // 2
    inv_freq = 1.0 / (THETA ** (np.arange(half, dtype=np.float64) * 2.0 / D))
    freqs = pos[:, None] * inv_freq[None, :]          # [T, 64]
    cos = np.cos(freqs).astype(np.float32).T          # [64, T]
    sin = np.sin(freqs).astype(np.float32).T
    cosT = np.ascontiguousarray(
        np.concatenate([cos, cos], axis=0)).astype(BFNP)    # [128, T]
    sinT = np.ascontiguousarray(
        np.concatenate([-sin, sin], axis=0)).astype(BFNP)   # sign fold
    hs = np.asarray(hidden_states, dtype=np.float32)
    # hid_r[n, p, kc, t] = hs[n*512+t, kc*128+p]
    hid_r = np.ascontiguousarray(
        hs.reshape(NCH, 512, KCH, P).transpose(0, 3, 2, 1)).astype(BFNP)
    w_qkv = np.asarray(w_qkv, dtype=np.float32)
    w_o = np.asarray(w_o, dtype=np.float32)

    in_maps = []
    for c in range(CORES):
        rows = [
            w_qkv[c * QH * D:(c + 1) * QH * D],                         # 4 q heads
            w_qkv[H * D + c * D: H * D + (c + 1) * D],                  # k head
            w_qkv[(H + KV) * D + c * D: (H + KV) * D + (c + 1) * D],    # v head
        ]
        wq_c = np.concatenate(rows, axis=0)                  # [768, 4096]
        # w_qkv_r[p, kc, m] = wq_c[m, kc*128+p]
        w_qkv_r = np.ascontiguousarray(
            wq_c.reshape(SLOTS * P, KCH, P).transpose(2, 1, 0)).astype(BFNP)
        wo_cT = w_o[c * LOCAL:(c + 1) * LOCAL, :].T          # [4096, 512]
        # w_o_r[p, kc, m] = wo_cT[kc*128+p, m]
        w_o_r = np.ascontiguousarray(
            wo_cT.reshape(KCH, P, LOCAL).transpose(1, 0, 2)).astype(BFNP)
        in_maps.append({
            "hid_r": hid_r,
            "w_qkv_r": w_qkv_r,
            "w_o_r": w_o_r,
            "cosT": cosT,
            "sinT": sinT,
        })
    return in_maps


def kernel(positions, hidden_states, w_qkv, w_o):
    global _NC_CACHE
    in_maps = _host_prep(positions, hidden_states, w_qkv, w_o)
    if _NC_CACHE is None:
        _NC_CACHE = build_nc()
    res = None
    for attempt in range(3):
        try:
            res = run_bass_kernel_spmd(_NC_CACHE, in_maps,
                                       core_ids=list(range(CORES)))
            break
        except Exception:
            if attempt == 2:
                raise
    outs = [res.results[c]["outT"].T for c in range(CORES)]   # [2048, 512] each
    return np.ascontiguousarray(np.concatenate(outs, axis=1))


# revision 48
# speedup vs baseline: 1.0925x; 1.0666x over previous
"""Mixtral-style GQA attention block on 8 Trainium2 NeuronCores.

Tensor-parallel over heads: core c owns q-heads [4c..4c+4) and kv-head c.
bf16 datapath throughout (2e-2 rel-err budget allows it): halves DMA
traffic, doubles DVE elementwise rate, enables FWL weight loads.

Per core:
  phase 1 (n-outer over 512-token chunks): qkv proj with full 32-step
    PSUM accumulation (no SBUF adds) -> RoPE (PE permutation rotate)
    -> v transpose.
  phase 2: causal attention per chunk (transposed-scores layout, shared
    stationaries), softmax-normalize, per-chunk bf16 AllGather of the
    normalized AV, and o_proj for this core's 512 output columns emitted
    per chunk so it overlaps later chunks' attention.
Host concatenates the per-core column slices.

Model dims (hardcoded): T=2048, HIDDEN=4096, H=32, KV=8, D=128.
"""

from contextlib import ExitStack

import numpy as np
import ml_dtypes

import concourse.bass_utils as _bu
import concourse.mybir as mybir
import concourse.tile as tile
from concourse import bacc
from concourse.bass_utils import run_bass_kernel_spmd

# ---- problem dims ----
T = 2048
HIDDEN = 4096
H = 32
KV = 8
D = 128
THETA = 10000.0
SCALE = D ** -0.5

CORES = 8
QH = H // CORES            # 4 q heads per core
SLOTS = QH + 2             # q0..q3, k, v head-major slots
LOCAL = QH * D             # 512: per-core attention output dims
P = 128
NCH = T // 512             # 4 token chunks of 512
KCH = HIDDEN // P          # 32 contraction chunks
TT = T // P                # 16 token tiles of 128

BF16 = mybir.dt.bfloat16
F32 = mybir.dt.float32
F32R = mybir.dt.float32r
EXP = mybir.ActivationFunctionType.Exp
BFNP = ml_dtypes.bfloat16


def build_nc():
    nc = bacc.Bacc(num_devices=CORES)

    # ---- per-core I/O (host pre-laid-out for contiguous DMA) ----
    hid_r = nc.declare_dram_parameter("hid_r", [NCH, P, KCH, 512], BF16,
                                      isOutput=False)
    w_qkv_r = nc.declare_dram_parameter("w_qkv_r", [P, KCH, SLOTS * P], BF16,
                                        isOutput=False)
    w_o_r = nc.declare_dram_parameter("w_o_r", [P, KCH, LOCAL], BF16,
                                      isOutput=False)
    cosT = nc.declare_dram_parameter("cosT", [P, T], BF16, isOutput=False)
    sinT = nc.declare_dram_parameter("sinT", [P, T], BF16, isOutput=False)
    outT = nc.declare_dram_parameter("outT", [LOCAL, T], F32, isOutput=True)

    # ---- consts ----
    rotm = np.zeros((P, P), dtype=np.float32)
    rotm[np.arange(P), (np.arange(P) + 64) % P] = 1.0   # rot(x)[i] = x[(i+64)%128]
    rot_c = nc.inline_tensor(
        np.ascontiguousarray(rotm.T).astype(BFNP), name="rot_c")
    eye_c = nc.inline_tensor(np.eye(P, dtype=np.float32).astype(BFNP),
                             name="eye_c")
    tri_c = nc.inline_tensor(
        np.triu(np.ones((P, P), dtype=np.float32)).astype(BFNP), name="tri_c")
    # dn lhsT for head h: [128, 4] with column h all-ones
    onc4 = np.zeros((P, QH, QH), dtype=np.float32)
    for h in range(QH):
        onc4[:, h, h] = 1.0
    onc4_c = nc.inline_tensor(
        np.ascontiguousarray(onc4.transpose(1, 0, 2)).astype(BFNP),
        name="onc4_c")   # [QH, 128, 4]
    ones_row_c = nc.inline_tensor(np.ones((1, P), dtype=np.float32).astype(BFNP),
                                  name="ones_row_c")

    # ---- collective bounce buffers (chunk-major, bf16) ----
    ag_in = nc.dram_tensor("ag_in", [NCH, LOCAL, 512], BF16)
    ag_out = nc.dram_tensor("ag_out", [NCH, H * D, 512], BF16,
                            addr_space="Shared")


    with tile.TileContext(nc) as tc:
        with tc.tile_pool(name="const", bufs=1) as cpool:
            qstack = ExitStack()
            qpool = qstack.enter_context(tc.tile_pool(name="qkv_out", bufs=1))
            rot_sb = cpool.tile([P, P], BF16, tag="rot")
            eye_sb = cpool.tile([P, P], BF16, tag="eye")
            tri_sb = cpool.tile([P, P], BF16, tag="tri")
            onc4_sb = [cpool.tile([P, QH], BF16, tag=f"onc4_{h}",
                                  name=f"onc4_{h}")
                       for h in range(QH)]
            onrb_sb = cpool.tile([1, P], BF16, tag="onrb")
            nc.sync.dma_start(rot_sb[:], rot_c[:, :])
            nc.sync.dma_start(eye_sb[:], eye_c[:, :])
            nc.sync.dma_start(tri_sb[:], tri_c[:, :])
            for h in range(QH):
                nc.sync.dma_start(onc4_sb[h][:], onc4_c[h])
            nc.sync.dma_start(onrb_sb[:], ones_row_c[:, :])


            # persistent qkv outputs (head-major); rope applied in place
            qkv_sb = [qpool.tile([P, T], BF16, tag=f"qkv{m}", name=f"qkv{m}")
                      for m in range(SLOTS)]
            q_rope = qkv_sb[:QH]
            k_rope = qkv_sb[QH]
            vtok = [qpool.tile([P, P], BF16, tag=f"vt{j}", name=f"vt{j}")
                    for j in range(TT)]

            # ============ phase 1: qkv proj (full-psum accum) + rope ====
            # kc-outer / m-inner over 6 concurrent PSUM banks: PE starts as
            # soon as the first contraction chunk lands, instead of waiting
            # for the full weight+hid preload.
            with tc.tile_pool(name="wq", bufs=1) as wq_pool, \
                 tc.tile_pool(name="hid", bufs=2) as hid_pool, \
                 tc.tile_pool(name="cs", bufs=1) as cs_pool, \
                 tc.tile_pool(name="ev", bufs=4) as ev_pool, \
                 tc.tile_pool(name="qkv_ps", bufs=1, space="PSUM") as qkv_ps, \
                 tc.tile_pool(name="rot_ps", bufs=2, space="PSUM") as rot_ps:
                wq = wq_pool.tile([P, KCH, SLOTS * P], BF16, tag="wq")
                for kc in range(KCH):
                    nc.sync.dma_start(wq[:, kc, :], w_qkv_r[:, kc, :])
                cos_sb = cs_pool.tile([P, T], BF16, tag="cos")
                sin_sb = cs_pool.tile([P, T], BF16, tag="sin")
                for n in range(NCH):
                    nc.sync.dma_start(cos_sb[:, n * 512:(n + 1) * 512],
                                      cosT[:, n * 512:(n + 1) * 512])
                    nc.sync.dma_start(sin_sb[:, n * 512:(n + 1) * 512],
                                      sinT[:, n * 512:(n + 1) * 512])

                for n in range(NCH):
                    t0 = n * 512
                    ht = hid_pool.tile([P, KCH, 512], BF16, tag="ht",
                                       name="ht")
                    for kc in range(KCH):
                        nc.scalar.dma_start(ht[:, kc, :],
                                            hid_r[n, :, kc, :])
                    pss = [qkv_ps.tile([P, 512], F32, tag=f"qp{m}",
                                       name=f"qp{m}")
                           for m in range(SLOTS)]
                    for kc in range(KCH):
                        for m in range(SLOTS):
                            nc.tensor.matmul(
                                pss[m][:], wq[:, kc, m * P:(m + 1) * P],
                                ht[:, kc, :],
                                start=(kc == 0), stop=(kc == KCH - 1),
                                skip_group_check=True)
                    for m in range(SLOTS):
                        dslc = qkv_sb[m][:, t0:t0 + 512]
                        nc.vector.tensor_copy(dslc, pss[m][:])
                        if m < SLOTS - 1:
                            # neox rope via PE permutation rotate
                            rps = rot_ps.tile([P, 512], F32, tag="rot",
                                              name="rps")
                            nc.tensor.matmul(rps[:], rot_sb[:], dslc,
                                             start=True, stop=True)
                            tmp = ev_pool.tile([P, 512], BF16, tag="tmp")
                            nc.vector.tensor_mul(tmp[:], rps[:],
                                                 sin_sb[:, t0:t0 + 512])
                            nc.vector.tensor_mul(dslc, dslc,
                                                 cos_sb[:, t0:t0 + 512])
                            nc.vector.tensor_add(dslc, dslc, tmp[:])
                        else:
                            # v: transpose to [keys, D] token tiles
                            for jj in range(4):
                                j = n * 4 + jj
                                # bf16 [P,1024] = same 2KB bank footprint as
                                # f32 [P,512]; transpose out dtype must match
                                # its bf16 input
                                tps = rot_ps.tile([P, 1024], BF16, tag="rot",
                                                  name="tps")
                                nc.tensor.transpose(
                                    tps[:, :P],
                                    qkv_sb[SLOTS - 1][:, j * P:(j + 1) * P],
                                    eye_sb[:])
                                nc.scalar.copy(vtok[j][:], tps[:, :P])

            # ============ phase 2: attention + chunked AllGather + o_proj ====
            with tc.tile_pool(name="wo", bufs=1) as wo_pool, \
                 tc.tile_pool(name="att", bufs=1) as att_pool, \
                 tc.tile_pool(name="sm", bufs=2) as sm_pool, \
                 tc.tile_pool(name="agr", bufs=3) as ag_pool, \
                 tc.tile_pool(name="ob", bufs=2) as out_pool, \
                 tc.tile_pool(name="sc_ps", bufs=2, space="PSUM") as sc_ps, \
                 tc.tile_pool(name="av_ps", bufs=1, space="PSUM") as av_ps, \
                 tc.tile_pool(name="dn_ps", bufs=1, space="PSUM") as dn_ps, \
                 tc.tile_pool(name="op_ps", bufs=1, space="PSUM") as op_ps:
                # wo + ag reads ride the GpSimd queue: an ag read waits on
                # its AllGather's completion, and a waiting DMA blocks the
                # whole issuing engine's FIFO — on Sync that wedged the
                # norm/dnr DMAs of later chunks behind it.
                wo_sb = wo_pool.tile([P, KCH, LOCAL], BF16, tag="wo")
                for kc in range(KCH):
                    nc.gpsimd.dma_start(wo_sb[:, kc, :], w_o_r[:, kc, :])

                def make_oproj(c):
                    # split into pieces so the PE-queue (strict MATMUL FIFO)
                    # never holds an oproj MM that still waits on its
                    # AllGather: pieces are emitted two chunks later, when
                    # AG(c) has long completed.
                    state = {}

                    def load_ag():
                        ag = ag_pool.tile([P, KCH, 512], BF16, tag="ag",
                                          name="ag")
                        for kb in range(KCH // 4):
                            nc.gpsimd.dma_start(
                                ag[:, kb * 4:(kb + 1) * 4, :],
                                ag_out[c, kb * 512:(kb + 1) * 512, :]
                                .rearrange("(kc p) t -> p kc t", p=P))
                        state["ag"] = ag

                    def make_m(m):
                        def mpiece():
                            ag = state["ag"]
                            ps = op_ps.tile([P, 512], F32, tag="op",
                                            name="op")
                            for kc in range(KCH):
                                nc.tensor.matmul(
                                    ps[:], wo_sb[:, kc, m * P:(m + 1) * P],
                                    ag[:, kc, :],
                                    start=(kc == 0), stop=(kc == KCH - 1),
                                    skip_group_check=True)
                            ob = out_pool.tile([P, 512], F32, tag="ob")
                            nc.vector.tensor_copy(ob[:], ps[:])
                            nc.sync.dma_start(
                                outT[m * P:(m + 1) * P,
                                     c * 512:(c + 1) * 512], ob[:])
                        return mpiece

                    return load_ag, [make_m(m) for m in range(LOCAL // P)]

                oproj = []

                norm_pending = None
                for c in range(NCH):
                    t0 = c * 512
                    jmax = 4 * c + 3
                    avp = [av_ps.tile([P, 512], F32, tag=f"av{h}",
                                      name=f"av{h}")
                           for h in range(QH)]
                    dnp = dn_ps.tile([QH, 512], F32, tag="dn")
                    atts = {}

                    def scores(j, c=c, t0=t0, atts=atts):
                        toff = max(t0, j * P)
                        w = t0 + 512 - toff
                        for h in range(QH):
                            scp = sc_ps.tile([P, 512], F32, tag="sc",
                                             name="scp")
                            nc.tensor.matmul(
                                scp[:, :w], k_rope[:, j * P:(j + 1) * P],
                                q_rope[h][:, toff:toff + w],
                                start=True, stop=True)
                            att = att_pool.tile([P, 512], BF16, tag="att",
                                                name="att", bufs=10)
                            nc.scalar.activation(att[:, :w], scp[:, :w], EXP,
                                                 scale=SCALE)
                            if j >= 4 * c:  # diagonal block: causal mask
                                nc.vector.tensor_mul(att[:, :P], att[:, :P],
                                                     tri_sb[:])
                            atts[(j, h)] = (att, toff, w)

                    def avdn(j, c=c, t0=t0, jmax=jmax, atts=atts, avp=avp,
                             dnp=dnp):
                        for h in range(QH):
                            att, toff, w = atts[(j, h)]
                            o = toff - t0
                            nc.tensor.matmul(
                                avp[h][:, o:o + w], vtok[j][:], att[:, :w],
                                start=(j == 0), stop=(j == jmax),
                                skip_group_check=True)
                        for h in range(QH):
                            att, toff, w = atts[(j, h)]
                            o = toff - t0
                            nc.tensor.matmul(
                                dnp[:, o:o + w], onc4_sb[h][:], att[:, :w],
                                start=(j == 0 and h == 0),
                                stop=(j == jmax and h == QH - 1),
                                skip_group_check=True)

                    def make_norm(c=c, avp=avp, dnp=dnp):
                        def norm():
                            dn_sb = sm_pool.tile([QH, 512], F32, tag="dn_sb")
                            nc.scalar.copy(dn_sb[:], dnp[:])
                            rc4 = sm_pool.tile([QH, 512], F32, tag="rc4")
                            scr = sm_pool.tile([QH, 512], F32, tag="scr")
                            nc.vector.reciprocal_approx_accurate(
                                rc4[:], dn_sb[:], scr[:])
                            # bf16 path keeps the broadcast MM single-pass
                            # (f32 runs the PE in two-pass LOW/HIGH mode)
                            rc4b = sm_pool.tile([QH, 512], BF16, tag="rc4b")
                            nc.vector.tensor_copy(rc4b[:], rc4[:])
                            # one partition-collapsing DMA for all 4 heads
                            dnr4 = sm_pool.tile([1, QH, 512], BF16,
                                                tag="dnr4", bufs=2)
                            nc.sync.dma_start(dnr4[:], rc4b[:])
                            for h in range(QH):
                                bcp = sc_ps.tile([P, 512], F32, tag="sc",
                                                 name="bcp")
                                nc.tensor.matmul(bcp[:], onrb_sb[:],
                                                 dnr4[0:1, h, :],
                                                 start=True, stop=True)
                                bc_sb = sm_pool.tile([P, 512], F32,
                                                     tag="bc_sb", bufs=4)
                                nc.scalar.copy(bc_sb[:], bcp[:])
                                avn = sm_pool.tile([P, 512], BF16, tag="avn",
                                                   bufs=4)
                                nc.vector.tensor_mul(avn[:], avp[h][:],
                                                     bc_sb[:])
                                nc.sync.dma_start(
                                    ag_in[c, h * P:(h + 1) * P, :], avn[:])
                            nc.gpsimd.collective_compute(
                                "AllGather",
                                mybir.AluOpType.bypass,
                                replica_groups=[list(range(CORES))],
                                ins=[ag_in[c]],
                                outs=[ag_out[c]],
                            )
                        return norm

                    # software-pipeline: scores one j ahead; chunk c-1's
                    # normalization fires at j==0. All o_proj work is packed
                    # at the very end of the attention stream: the AGs
                    # complete underneath it, and no PE matmul ever waits in
                    # the strict PE FIFO on an unfinished collective. ag
                    # loads are placed so they never sit in the gpsimd FIFO
                    # ahead of a later AG trigger (they would delay it).
                    scores(0)
                    for j in range(jmax + 1):
                        if j < jmax:
                            scores(j + 1)
                        if j == 0 and norm_pending is not None:
                            norm_pending()
                        if c == NCH - 1:
                            if j == 1:
                                oproj[0][0]()        # load ag(0)
                            elif j == 8:
                                oproj[1][0]()        # load ag(1)
                            elif j >= 12:
                                oproj[0][1][j - 12]()   # mp(0) pieces
                        avdn(j)
                    norm_pending = make_norm()
                    oproj.append(make_oproj(c))
                norm_pending()                        # issues AG(3)
                oproj[2][0]()                         # load ag(2)
                oproj[3][0]()                         # load ag(3): parks on
                                                      # AG(3), queue empty
                for c2 in (1, 2, 3):
                    for piece in oproj[c2][1]:
                        piece()

            qstack.close()

    nc.finalize()
    return nc


_NC_CACHE = None


def _host_prep(positions, hidden_states, w_qkv, w_o):
    pos = np.asarray(positions).astype(np.float64)
    half = D // 2
    inv_freq = 1.0 / (THETA ** (np.arange(half, dtype=np.float64) * 2.0 / D))
    freqs = pos[:, None] * inv_freq[None, :]          # [T, 64]
    cos = np.cos(freqs).astype(np.float32).T          # [64, T]
    sin = np.sin(freqs).astype(np.float32).T
    cosT = np.ascontiguousarray(
        np.concatenate([cos, cos], axis=0)).astype(BFNP)    # [128, T]
    sinT = np.ascontiguousarray(
        np.concatenate([-sin, sin], axis=0)).astype(BFNP)   # sign fold
    hs = np.asarray(hidden_states, dtype=np.float32)
    # hid_r[n, p, kc, t] = hs[n*512+t, kc*128+p]
    hid_r = np.ascontiguousarray(
        hs.reshape(NCH, 512, KCH, P).transpose(0, 3, 2, 1)).astype(BFNP)
    w_qkv = np.asarray(w_qkv, dtype=np.float32)
    w_o = np.asarray(w_o, dtype=np.float32)

    in_maps = []
    for c in range(CORES):
        rows = [
            w_qkv[c * QH * D:(c + 1) * QH * D],                         # 4 q heads
            w_qkv[H * D + c * D: H * D + (c + 1) * D],                  # k head
            w_qkv[(H + KV) * D + c * D: (H + KV) * D + (c + 1) * D],    # v head
        ]
        wq_c = np.concatenate(rows, axis=0)                  # [768, 4096]
        # w_qkv_r[p, kc, m] = wq_c[m, kc*128+p]
        w_qkv_r = np.ascontiguousarray(
            wq_c.reshape(SLOTS * P, KCH, P).transpose(2, 1, 0)).astype(BFNP)
        wo_cT = w_o[c * LOCAL:(c + 1) * LOCAL, :].T          # [4096, 512]
        # w_o_r[p, kc, m] = wo_cT[kc*128+p, m]
        w_o_r = np.ascontiguousarray(
            wo_cT.reshape(KCH, P, LOCAL).transpose(1, 0, 2)).astype(BFNP)
        in_maps.append({
            "hid_r": hid_r,
            "w_qkv_r": w_qkv_r,
            "w_o_r": w_o_r,
            "cosT": cosT,
            "sinT": sinT,
        })
    return in_maps


def kernel(positions, hidden_states, w_qkv, w_o):
    global _NC_CACHE
    in_maps = _host_prep(positions, hidden_states, w_qkv, w_o)
    if _NC_CACHE is None:
        _NC_CACHE = build_nc()
    res = None
    for attempt in range(3):
        try:
            res = run_bass_kernel_spmd(_NC_CACHE, in_maps,
                                       core_ids=list(range(CORES)))
            break
        except Exception:
            if attempt == 2:
                raise
    outs = [res.results[c]["outT"].T for c in range(CORES)]   # [2048, 512] each
    return np.ascontiguousarray(np.concatenate(outs, axis=1))


# revision 50
# speedup vs baseline: 1.1560x; 1.0581x over previous
"""Mixtral-style GQA attention block on 8 Trainium2 NeuronCores.

Tensor-parallel over heads: core c owns q-heads [4c..4c+4) and kv-head c.
bf16 datapath throughout (2e-2 rel-err budget allows it): halves DMA
traffic, doubles DVE elementwise rate, enables FWL weight loads.

Per core:
  phase 1 (n-outer over 512-token chunks): qkv proj with full 32-step
    PSUM accumulation (no SBUF adds) -> RoPE (PE permutation rotate)
    -> v transpose.
  phase 2: causal attention per chunk (transposed-scores layout, shared
    stationaries), softmax-normalize, per-chunk bf16 AllGather of the
    normalized AV, and o_proj for this core's 512 output columns emitted
    per chunk so it overlaps later chunks' attention.
Host concatenates the per-core column slices.

Model dims (hardcoded): T=2048, HIDDEN=4096, H=32, KV=8, D=128.
"""

from contextlib import ExitStack

import numpy as np
import ml_dtypes

import concourse.bass_utils as _bu
import concourse.mybir as mybir
import concourse.tile as tile
from concourse import bacc
from concourse.bass_utils import run_bass_kernel_spmd

# ---- problem dims ----
T = 2048
HIDDEN = 4096
H = 32
KV = 8
D = 128
THETA = 10000.0
SCALE = D ** -0.5

CORES = 8
QH = H // CORES            # 4 q heads per core
SLOTS = QH + 2             # q0..q3, k, v head-major slots
LOCAL = QH * D             # 512: per-core attention output dims
P = 128
NCH = T // 512             # 4 token chunks of 512
KCH = HIDDEN // P          # 32 contraction chunks
TT = T // P                # 16 token tiles of 128

BF16 = mybir.dt.bfloat16
F32 = mybir.dt.float32
F32R = mybir.dt.float32r
EXP = mybir.ActivationFunctionType.Exp
BFNP = ml_dtypes.bfloat16


def build_nc():
    nc = bacc.Bacc(num_devices=CORES)

    # ---- per-core I/O (host pre-laid-out for contiguous DMA) ----
    hid_r = nc.declare_dram_parameter("hid_r", [NCH, P, KCH, 512], BF16,
                                      isOutput=False)
    w_qkv_r = nc.declare_dram_parameter("w_qkv_r", [P, KCH, SLOTS * P], BF16,
                                        isOutput=False)
    w_o_r = nc.declare_dram_parameter("w_o_r", [P, KCH, LOCAL], BF16,
                                      isOutput=False)
    cosT = nc.declare_dram_parameter("cosT", [P, T], BF16, isOutput=False)
    sinT = nc.declare_dram_parameter("sinT", [P, T], BF16, isOutput=False)
    outT = nc.declare_dram_parameter("outT", [LOCAL, T], F32, isOutput=True)

    # ---- consts ----
    rotm = np.zeros((P, P), dtype=np.float32)
    rotm[np.arange(P), (np.arange(P) + 64) % P] = 1.0   # rot(x)[i] = x[(i+64)%128]
    rot_c = nc.inline_tensor(
        np.ascontiguousarray(rotm.T).astype(BFNP), name="rot_c")
    eye_c = nc.inline_tensor(np.eye(P, dtype=np.float32).astype(BFNP),
                             name="eye_c")
    tri_c = nc.inline_tensor(
        np.triu(np.ones((P, P), dtype=np.float32)).astype(BFNP), name="tri_c")
    # dn lhsT for head h: [128, 4] with column h all-ones
    onc4 = np.zeros((P, QH, QH), dtype=np.float32)
    for h in range(QH):
        onc4[:, h, h] = 1.0
    onc4_c = nc.inline_tensor(
        np.ascontiguousarray(onc4.transpose(1, 0, 2)).astype(BFNP),
        name="onc4_c")   # [QH, 128, 4]
    ones_row_c = nc.inline_tensor(np.ones((1, P), dtype=np.float32).astype(BFNP),
                                  name="ones_row_c")

    # ---- collective bounce buffers (chunk-major, bf16) ----
    ag_in = nc.dram_tensor("ag_in", [NCH, LOCAL, 512], BF16)
    ag_out = nc.dram_tensor("ag_out", [NCH, H * D, 512], BF16,
                            addr_space="Shared")


    with tile.TileContext(nc) as tc:
        with tc.tile_pool(name="const", bufs=1) as cpool:
            qstack = ExitStack()
            qpool = qstack.enter_context(tc.tile_pool(name="qkv_out", bufs=1))
            rot_sb = cpool.tile([P, P], BF16, tag="rot")
            eye_sb = cpool.tile([P, P], BF16, tag="eye")
            tri_sb = cpool.tile([P, P], BF16, tag="tri")
            onc4_sb = [cpool.tile([P, QH], BF16, tag=f"onc4_{h}",
                                  name=f"onc4_{h}")
                       for h in range(QH)]
            onrb_sb = cpool.tile([1, P], BF16, tag="onrb")
            nc.sync.dma_start(rot_sb[:], rot_c[:, :])
            nc.sync.dma_start(eye_sb[:], eye_c[:, :])
            nc.sync.dma_start(tri_sb[:], tri_c[:, :])
            for h in range(QH):
                nc.sync.dma_start(onc4_sb[h][:], onc4_c[h])
            nc.sync.dma_start(onrb_sb[:], ones_row_c[:, :])


            # persistent qkv outputs (head-major); rope applied in place
            qkv_sb = [qpool.tile([P, T], BF16, tag=f"qkv{m}", name=f"qkv{m}")
                      for m in range(SLOTS)]
            q_rope = qkv_sb[:QH]
            k_rope = qkv_sb[QH]
            vtok = [qpool.tile([P, P], BF16, tag=f"vt{j}", name=f"vt{j}")
                    for j in range(TT)]

            # ============ phase 1: qkv proj (full-psum accum) + rope ====
            # kc-outer / m-inner over 6 concurrent PSUM banks: PE starts as
            # soon as the first contraction chunk lands, instead of waiting
            # for the full weight+hid preload.
            with tc.tile_pool(name="wq", bufs=1) as wq_pool, \
                 tc.tile_pool(name="hid", bufs=2) as hid_pool, \
                 tc.tile_pool(name="cs", bufs=1) as cs_pool, \
                 tc.tile_pool(name="ev", bufs=4) as ev_pool, \
                 tc.tile_pool(name="qkv_ps", bufs=1, space="PSUM") as qkv_ps, \
                 tc.tile_pool(name="rot_ps", bufs=2, space="PSUM") as rot_ps:
                wq = wq_pool.tile([P, KCH, SLOTS * P], BF16, tag="wq")
                for kc in range(KCH):
                    nc.sync.dma_start(wq[:, kc, :], w_qkv_r[:, kc, :])
                cos_sb = cs_pool.tile([P, T], BF16, tag="cos")
                sin_sb = cs_pool.tile([P, T], BF16, tag="sin")
                for n in range(NCH):
                    nc.sync.dma_start(cos_sb[:, n * 512:(n + 1) * 512],
                                      cosT[:, n * 512:(n + 1) * 512])
                    nc.sync.dma_start(sin_sb[:, n * 512:(n + 1) * 512],
                                      sinT[:, n * 512:(n + 1) * 512])

                for n in range(NCH):
                    t0 = n * 512
                    ht = hid_pool.tile([P, KCH, 512], BF16, tag="ht",
                                       name="ht")
                    for kc in range(KCH):
                        nc.scalar.dma_start(ht[:, kc, :],
                                            hid_r[n, :, kc, :])
                    pss = [qkv_ps.tile([P, 512], F32, tag=f"qp{m}",
                                       name=f"qp{m}")
                           for m in range(SLOTS)]
                    for kc in range(KCH):
                        for m in range(SLOTS):
                            nc.tensor.matmul(
                                pss[m][:], wq[:, kc, m * P:(m + 1) * P],
                                ht[:, kc, :],
                                start=(kc == 0), stop=(kc == KCH - 1),
                                skip_group_check=True)
                    # all copies first (on the otherwise-idle Scalar engine)
                    # so the next chunk's matmuls get their PSUM banks back
                    # without queueing behind the rope muls on Vector
                    for m in range(SLOTS):
                        nc.scalar.copy(qkv_sb[m][:, t0:t0 + 512], pss[m][:])
                    for m in range(SLOTS - 1):
                        # neox rope via PE permutation rotate
                        dslc = qkv_sb[m][:, t0:t0 + 512]
                        rps = rot_ps.tile([P, 512], F32, tag="rot",
                                          name="rps")
                        nc.tensor.matmul(rps[:], rot_sb[:], dslc,
                                         start=True, stop=True)
                        tmp = ev_pool.tile([P, 512], BF16, tag="tmp")
                        nc.vector.tensor_mul(tmp[:], rps[:],
                                             sin_sb[:, t0:t0 + 512])
                        nc.vector.tensor_mul(dslc, dslc,
                                             cos_sb[:, t0:t0 + 512])
                        nc.vector.tensor_add(dslc, dslc, tmp[:])
                    # v: transpose to [keys, D] token tiles
                    for jj in range(4):
                        j = n * 4 + jj
                        # bf16 [P,1024] = same 2KB bank footprint as
                        # f32 [P,512]; transpose out dtype must match
                        # its bf16 input
                        tps = rot_ps.tile([P, 1024], BF16, tag="rot",
                                          name="tps")
                        nc.tensor.transpose(
                            tps[:, :P],
                            qkv_sb[SLOTS - 1][:, j * P:(j + 1) * P],
                            eye_sb[:])
                        nc.scalar.copy(vtok[j][:], tps[:, :P])

            # ============ phase 2: attention + chunked AllGather + o_proj ====
            with tc.tile_pool(name="wo", bufs=1) as wo_pool, \
                 tc.tile_pool(name="att", bufs=1) as att_pool, \
                 tc.tile_pool(name="sm", bufs=2) as sm_pool, \
                 tc.tile_pool(name="agr", bufs=3) as ag_pool, \
                 tc.tile_pool(name="ob", bufs=2) as out_pool, \
                 tc.tile_pool(name="sc_ps", bufs=2, space="PSUM") as sc_ps, \
                 tc.tile_pool(name="av_ps", bufs=1, space="PSUM") as av_ps, \
                 tc.tile_pool(name="dn_ps", bufs=1, space="PSUM") as dn_ps, \
                 tc.tile_pool(name="op_ps", bufs=1, space="PSUM") as op_ps:
                # wo + ag reads ride the GpSimd queue: an ag read waits on
                # its AllGather's completion, and a waiting DMA blocks the
                # whole issuing engine's FIFO — on Sync that wedged the
                # norm/dnr DMAs of later chunks behind it.
                wo_sb = wo_pool.tile([P, KCH, LOCAL], BF16, tag="wo")
                for kc in range(KCH):
                    nc.gpsimd.dma_start(wo_sb[:, kc, :], w_o_r[:, kc, :])

                def make_oproj(c):
                    # split into pieces so the PE-queue (strict MATMUL FIFO)
                    # never holds an oproj MM that still waits on its
                    # AllGather: pieces are emitted two chunks later, when
                    # AG(c) has long completed.
                    state = {}

                    def load_ag():
                        ag = ag_pool.tile([P, KCH, 512], BF16, tag="ag",
                                          name="ag")
                        for kb in range(KCH // 4):
                            nc.gpsimd.dma_start(
                                ag[:, kb * 4:(kb + 1) * 4, :],
                                ag_out[c, kb * 512:(kb + 1) * 512, :]
                                .rearrange("(kc p) t -> p kc t", p=P))
                        state["ag"] = ag

                    def make_m(m):
                        def mpiece():
                            ag = state["ag"]
                            # ping-pong alternate m-groups onto the sc pool
                            # (idle once attention drains) so consecutive
                            # accumulation groups don't single-buffer on one
                            # bank behind their copy-out
                            if m % 2 == 0:
                                ps = op_ps.tile([P, 512], F32, tag="op",
                                                name="op")
                            else:
                                ps = sc_ps.tile([P, 512], F32, tag="sc",
                                                name="op_sc")
                            for kc in range(KCH):
                                nc.tensor.matmul(
                                    ps[:], wo_sb[:, kc, m * P:(m + 1) * P],
                                    ag[:, kc, :],
                                    start=(kc == 0), stop=(kc == KCH - 1),
                                    skip_group_check=True)
                            ob = out_pool.tile([P, 512], F32, tag="ob")
                            nc.vector.tensor_copy(ob[:], ps[:])
                            nc.sync.dma_start(
                                outT[m * P:(m + 1) * P,
                                     c * 512:(c + 1) * 512], ob[:])
                        return mpiece

                    return load_ag, [make_m(m) for m in range(LOCAL // P)]

                oproj = []

                norm_pending = None
                for c in range(NCH):
                    t0 = c * 512
                    jmax = 4 * c + 3
                    avp = [av_ps.tile([P, 512], F32, tag=f"av{h}",
                                      name=f"av{h}")
                           for h in range(QH)]
                    dnp = dn_ps.tile([QH, 512], F32, tag="dn")
                    atts = {}

                    def scores(j, c=c, t0=t0, atts=atts):
                        toff = max(t0, j * P)
                        w = t0 + 512 - toff
                        for h in range(QH):
                            scp = sc_ps.tile([P, 512], F32, tag="sc",
                                             name="scp")
                            nc.tensor.matmul(
                                scp[:, :w], k_rope[:, j * P:(j + 1) * P],
                                q_rope[h][:, toff:toff + w],
                                start=True, stop=True)
                            att = att_pool.tile([P, 512], BF16, tag="att",
                                                name="att", bufs=10)
                            nc.scalar.activation(att[:, :w], scp[:, :w], EXP,
                                                 scale=SCALE)
                            if j >= 4 * c:  # diagonal block: causal mask
                                nc.vector.tensor_mul(att[:, :P], att[:, :P],
                                                     tri_sb[:])
                            atts[(j, h)] = (att, toff, w)

                    def avdn(j, c=c, t0=t0, jmax=jmax, atts=atts, avp=avp,
                             dnp=dnp):
                        for h in range(QH):
                            att, toff, w = atts[(j, h)]
                            o = toff - t0
                            nc.tensor.matmul(
                                avp[h][:, o:o + w], vtok[j][:], att[:, :w],
                                start=(j == 0), stop=(j == jmax),
                                skip_group_check=True)
                        for h in range(QH):
                            att, toff, w = atts[(j, h)]
                            o = toff - t0
                            nc.tensor.matmul(
                                dnp[:, o:o + w], onc4_sb[h][:], att[:, :w],
                                start=(j == 0 and h == 0),
                                stop=(j == jmax and h == QH - 1),
                                skip_group_check=True)

                    def make_norm(c=c, avp=avp, dnp=dnp):
                        def norm():
                            dn_sb = sm_pool.tile([QH, 512], F32, tag="dn_sb")
                            nc.scalar.copy(dn_sb[:], dnp[:])
                            rc4 = sm_pool.tile([QH, 512], F32, tag="rc4")
                            scr = sm_pool.tile([QH, 512], F32, tag="scr")
                            nc.vector.reciprocal_approx_accurate(
                                rc4[:], dn_sb[:], scr[:])
                            # bf16 path keeps the broadcast MM single-pass
                            # (f32 runs the PE in two-pass LOW/HIGH mode)
                            rc4b = sm_pool.tile([QH, 512], BF16, tag="rc4b")
                            nc.vector.tensor_copy(rc4b[:], rc4[:])
                            # one partition-collapsing DMA for all 4 heads
                            dnr4 = sm_pool.tile([1, QH, 512], BF16,
                                                tag="dnr4", bufs=2)
                            nc.sync.dma_start(dnr4[:], rc4b[:])
                            for h in range(QH):
                                bcp = sc_ps.tile([P, 512], F32, tag="sc",
                                                 name="bcp")
                                nc.tensor.matmul(bcp[:], onrb_sb[:],
                                                 dnr4[0:1, h, :],
                                                 start=True, stop=True)
                                bc_sb = sm_pool.tile([P, 512], F32,
                                                     tag="bc_sb", bufs=4)
                                nc.scalar.copy(bc_sb[:], bcp[:])
                                avn = sm_pool.tile([P, 512], BF16, tag="avn",
                                                   bufs=4)
                                nc.vector.tensor_mul(avn[:], avp[h][:],
                                                     bc_sb[:])
                                nc.sync.dma_start(
                                    ag_in[c, h * P:(h + 1) * P, :], avn[:])
                            nc.gpsimd.collective_compute(
                                "AllGather",
                                mybir.AluOpType.bypass,
                                replica_groups=[list(range(CORES))],
                                ins=[ag_in[c]],
                                outs=[ag_out[c]],
                            )
                        return norm

                    # software-pipeline: scores one j ahead; chunk c-1's
                    # normalization fires at j==0. All o_proj work is packed
                    # at the very end of the attention stream: the AGs
                    # complete underneath it, and no PE matmul ever waits in
                    # the strict PE FIFO on an unfinished collective. ag
                    # loads are placed so they never sit in the gpsimd FIFO
                    # ahead of a later AG trigger (they would delay it).
                    scores(0)
                    for j in range(jmax + 1):
                        if j < jmax:
                            scores(j + 1)
                        if j == 0 and norm_pending is not None:
                            norm_pending()
                        if c == NCH - 1:
                            if j == 1:
                                oproj[0][0]()        # load ag(0)
                            elif j == 8:
                                oproj[1][0]()        # load ag(1)
                            elif j >= 12:
                                oproj[0][1][j - 12]()   # mp(0) pieces
                        avdn(j)
                    norm_pending = make_norm()
                    oproj.append(make_oproj(c))
                norm_pending()                        # issues AG(3)
                oproj[2][0]()                         # load ag(2)
                oproj[3][0]()                         # load ag(3): parks on
                                                      # AG(3), queue empty
                for c2 in (1, 2, 3):
                    for piece in oproj[c2][1]:
                        piece()

            qstack.close()

    nc.finalize()
    return nc


_NC_CACHE = None


def _host_prep(positions, hidden_states, w_qkv, w_o):
    pos = np.asarray(positions).astype(np.float64)
    half = D // 2
    inv_freq = 1.0 / (THETA ** (np.arange(half, dtype=np.float64) * 2.0 / D))
    freqs = pos[:, None] * inv_freq[None, :]          # [T, 64]
    cos = np.cos(freqs).astype(np.float32).T          # [64, T]
    sin = np.sin(freqs).astype(np.float32).T
    cosT = np.ascontiguousarray(
        np.concatenate([cos, cos], axis=0)).astype(BFNP)    # [128, T]
    sinT = np.ascontiguousarray(
        np.concatenate([-sin, sin], axis=0)).astype(BFNP)   # sign fold
    hs = np.asarray(hidden_states, dtype=np.float32)
    # hid_r[n, p, kc, t] = hs[n*512+t, kc*128+p]
    hid_r = np.ascontiguousarray(
        hs.reshape(NCH, 512, KCH, P).transpose(0, 3, 2, 1)).astype(BFNP)
    w_qkv = np.asarray(w_qkv, dtype=np.float32)
    w_o = np.asarray(w_o, dtype=np.float32)

    in_maps = []
    for c in range(CORES):
        rows = [
            w_qkv[c * QH * D:(c + 1) * QH * D],                         # 4 q heads
            w_qkv[H * D + c * D: H * D + (c + 1) * D],                  # k head
            w_qkv[(H + KV) * D + c * D: (H + KV) * D + (c + 1) * D],    # v head
        ]
        wq_c = np.concatenate(rows, axis=0)                  # [768, 4096]
        # w_qkv_r[p, kc, m] = wq_c[m, kc*128+p]
        w_qkv_r = np.ascontiguousarray(
            wq_c.reshape(SLOTS * P, KCH, P).transpose(2, 1, 0)).astype(BFNP)
        wo_cT = w_o[c * LOCAL:(c + 1) * LOCAL, :].T          # [4096, 512]
        # w_o_r[p, kc, m] = wo_cT[kc*128+p, m]
        w_o_r = np.ascontiguousarray(
            wo_cT.reshape(KCH, P, LOCAL).transpose(1, 0, 2)).astype(BFNP)
        in_maps.append({
            "hid_r": hid_r,
            "w_qkv_r": w_qkv_r,
            "w_o_r": w_o_r,
            "cosT": cosT,
            "sinT": sinT,
        })
    return in_maps


def kernel(positions, hidden_states, w_qkv, w_o):
    global _NC_CACHE
    in_maps = _host_prep(positions, hidden_states, w_qkv, w_o)
    if _NC_CACHE is None:
        _NC_CACHE = build_nc()
    res = None
    for attempt in range(3):
        try:
            res = run_bass_kernel_spmd(_NC_CACHE, in_maps,
                                       core_ids=list(range(CORES)))
            break
        except Exception:
            if attempt == 2:
                raise
    outs = [res.results[c]["outT"].T for c in range(CORES)]   # [2048, 512] each
    return np.ascontiguousarray(np.concatenate(outs, axis=1))
